# revision 53
# baseline (speedup 1.0000x reference)
"""Trainium2 Bass kernel for nn_DFVAE (3-stage MoE routing with sorted ids).

Static chunk-grid strategy (N=16384, LD=512, experts (8,6,16), 8 cores,
contiguous row shards, bf16 end-to-end):
  - Per (core, stage): 4 STATIC 512-row chunk windows at rows 512j (expert =
    run covering the chunk start), plus F_s dynamic 256-wide "fix" windows
    that rewrite rows between an unaligned run start and the next chunk
    boundary (<=2 fixes per boundary).  Fix windows may spill past row 2048
    into a 256-row pad of the activation tile (memset once, never stored).
  - Weights are host-packed PER CORE in window-slot order (bf16, lhsT
    layout), so every matmul lhsT is a STATIC SBUF address (PE lhsT cannot
    take register offsets).  Only fix windows use dynamic row offsets
    (values_load on PE/ACT/DVE).
  - Activations bf16 in two ping-pong SBUF tiles (A->B->A->B); static APs
    keep Tile's dependency tracking precise so DMA/compute pipeline.
  - z loaded in 4 per-chunk DMAs (pipelined head); output stored per chunk.
  - PSUM evacuation (bias add + relu) split between ACT (m even) and DVE
    (m odd).
"""
import numpy as np
import ml_dtypes

import concourse.mybir as mybir
import concourse.tile as tile
from concourse import bacc, bass_utils
from concourse.bass import ds

N = 16384
LD = 512
NCORES = 8
SH = N // NCORES      # 2048 rows per core
P = 128
KO = LD // P          # 4 contraction/feature subtiles
CH = 512              # static chunk rows
NCH = SH // CH        # 4 chunks per core
FIXW = 256            # fix window rows
PAD = 256             # activation tile pad rows (fix spill)
STAGE_E = (8, 6, 16)

BF16 = ml_dtypes.bfloat16

LAST_RESULTS = None  # test harness reads exec_time_ns off this

_program_cache = {}


def _core_fixes(loc, hazard=None):
    """Fix list [(start, expert)...] for one core's id vector.

    A single fix spills up to FIXW rows past its run; that is only rewritten
    by a later fix when the following run starts unaligned.  A run that ends
    exactly on a chunk boundary (next run aligned, no fix) would leave the
    spill corrupt -- flag it so the caller can fall back.
    """
    starts = np.flatnonzero(np.diff(loc)) + 1
    fl = []
    for i, bp in enumerate(starts):
        bp = int(bp)
        if bp % CH == 0:
            continue
        run_end = int(starts[i + 1]) if i + 1 < len(starts) else SH
        cover_end = min(run_end, (bp // CH + 1) * CH)
        if cover_end <= bp:
            continue
        e = int(loc[bp])
        ln = cover_end - bp
        if ln <= FIXW:
            fl.append((bp, e))
            if (hazard is not None and ln < FIXW and run_end % CH == 0
                    and run_end < SH):
                hazard.append(bp)
        else:
            fl.append((bp, e))
            fl.append((cover_end - FIXW, e))
    fl.sort()
    return fl


def _assign_rows(ids_all):
    """Order-preserving row->core assignment that steers breakpoint offsets.

    Cores are built from the global stream; core c may defer a tail slice of
    its first atomic block to core c+1 (prepended there).  Removing g rows
    early shifts every later breakpoint left by g (mod 512), turning 2-fix
    boundaries (offset < 256) into 1-fix ones; the deferred rows create one
    junction boundary in the next core, whose offset we also control.
    """
    trip = ids_all[0].astype(np.int64) * 10000 + ids_all[1] * 100 + ids_all[2]
    block_of = np.cumsum(np.diff(trip, prepend=trip[0]) != 0)

    # beam over (donor block, donation size); score = per-stage max fixes
    # across cores (the SPMD shape cost), then total fixes
    beam = [(0, 0, np.empty(0, np.int64), (0, 0, 0), 0, [])]
    for c in range(NCORES):
        nxt_states = []
        for _, cursor, bag, fmax, ftot, rs in beam:
            b = len(bag)
            # candidate donations: (hi, g) = remove rows [hi-g, hi) where
            # [.., hi) is the in-range tail of some atomic block
            cands = {(0, 0)}
            if c < NCORES - 1:
                base_end = cursor + SH - b
                # block portions fully inside the base range
                blks = block_of[cursor:base_end]
                ends = np.flatnonzero(np.diff(blks)) + 1  # block ends (local)
                los = np.concatenate([[0], ends])
                his = np.concatenate([ends, [SH - b]])
                # breakpoint repair targets from the unshifted layout
                loc0 = np.concatenate([bag, np.arange(cursor, base_end)])
                wants = set()
                for s in range(3):
                    loc = ids_all[s][loc0]
                    for bp in (np.flatnonzero(np.diff(loc)) + 1):
                        off = int(bp) % CH
                        for tgt in (0, FIXW, 384):
                            gg = (tgt - off) % CH
                            if gg:
                                wants.add((int(bp), gg))
                for lo, hi in zip(los.tolist(), his.tolist()):
                    avail = hi - lo
                    glo = cursor + hi  # global end of this block portion
                    for bp, gg in wants:
                        # donor at/before the repaired breakpoint
                        if gg < avail and b + hi <= bp + gg:
                            cands.add((glo, gg))
                    if avail > 256:
                        cands.add((glo, 256))
            for hi, g in sorted(cands):
                take = SH - b + g
                if cursor + take > N or (c == NCORES - 1 and
                                         cursor + take != N):
                    continue
                if g:
                    idx = np.concatenate([
                        bag,
                        np.arange(cursor, hi - g),
                        np.arange(hi, cursor + take),
                    ])
                else:
                    idx = np.concatenate([bag,
                                          np.arange(cursor, cursor + take)])
                if len(idx) != SH:
                    continue
                fcs = [len(_core_fixes(ids_all[s][idx])) for s in range(3)]
                nf = (max(fmax[0], fcs[0]), max(fmax[1], fcs[1]),
                      max(fmax[2], fcs[2]))
                key = (nf[0] + nf[1] + nf[2], nf[2], ftot + sum(fcs))
                nbag = np.arange(hi - g, hi) if g else np.empty(0, np.int64)
                nxt_states.append((key, cursor + take, nbag,
                                   nf, ftot + sum(fcs), rs + [idx]))
        nxt_states.sort(key=lambda st: st[0])
        seen = set()
        beam = []
        zero_path = None
        for st in nxt_states:
            sk = (st[1], len(st[2]), int(st[2][0]) if len(st[2]) else -1)
            if st[1] == (c + 1) * SH and len(st[2]) == 0:
                zero_path = zero_path or st
            if sk in seen:
                continue
            seen.add(sk)
            beam.append(st)
            if len(beam) >= 24:
                break
        if zero_path is not None and zero_path not in beam:
            beam.append(zero_path)
    key, cursor, bag, fmax, ftot, rows = beam[0]
    assert cursor == N and len(bag) == 0, (cursor, len(bag))
    if sum(fmax) >= sum(
            max(len(_core_fixes(ids_all[s][c * SH:(c + 1) * SH]))
                for c in range(NCORES)) for s in range(3)):
        return None  # no better than contiguous
    return rows


def _structure(ids_all, rows=None):
    """Per stage: (chunk_experts[8][4], fixes[8]=[(start,expert)...], F)."""
    out = []
    for s in range(3):
        ids = ids_all[s]
        chunk_e = np.zeros((NCORES, NCH), np.int64)
        fixes = []
        for c in range(NCORES):
            loc = ids[rows[c]] if rows is not None else ids[c * SH:(c + 1) * SH]
            for j in range(NCH):
                chunk_e[c, j] = loc[j * CH]
            fixes.append(_core_fixes(loc))
        F = max(len(f) for f in fixes)
        for c in range(NCORES):
            fl = fixes[c]
            filler = fl[-1] if fl else (0, int(chunk_e[c][0]))
            while len(fl) < F:
                fl.append(filler)
        out.append((chunk_e, fixes, F))
    return out


def _pack_w(W):
    """[E, LD, LD] -> [E, P, KO*LD] lhsT layout (k-major blocks)."""
    E = W.shape[0]
    return np.ascontiguousarray(
        W.reshape(E, KO, P, LD).transpose(0, 2, 1, 3).reshape(E, P, KO * LD))


WARMUP = 24
GAPFILL = (0, 0, 0, 0)


def _build_program(F, has_bias=True):
    F0, F1, F2 = F
    S = [NCH + F0, NCH + F1, NCH + F2]
    S_tot = sum(S)
    F_tot = F0 + F1 + F2
    nc = bacc.Bacc("TRN2", target_bir_lowering=False, debug=False,
                   enable_asserts=False, num_devices=NCORES)
    bf = mybir.dt.bfloat16
    f32 = mybir.dt.float32
    i32 = mybir.dt.int32
    PE = mybir.EngineType.PE
    ACT = mybir.EngineType.Activation
    DVE = mybir.EngineType.DVE

    ND = max(F_tot, 1)
    zT = nc.dram_tensor("zT", [LD, SH], bf, kind="ExternalInput").ap()
    Wt = nc.dram_tensor("Wpk", [S_tot * P, KO * LD], bf, kind="ExternalInput").ap()
    Bt = (nc.dram_tensor("bias", [P, S_tot * KO], f32, kind="ExternalInput").ap()
          if has_bias else None)
    Dt = nc.dram_tensor("desc", [1, ND], i32, kind="ExternalInput").ap()
    Ot = nc.dram_tensor("outT", [LD, SH], bf, kind="ExternalOutput").ap()
    # stage-2 fix results land in disjoint static staging; host merges them
    Ft = nc.dram_tensor("fixO", [LD, max(F2, 1) * FIXW], bf,
                        kind="ExternalOutput").ap()

    zv = zT.rearrange("(ko p) r -> p ko r", p=P)
    ov = Ot.rearrange("(ko p) r -> p ko r", p=P)
    fv = Ft.rearrange("(ko p) r -> p ko r", p=P)
    Wv = Wt.rearrange("(s p) c -> s p c", p=P)

    soff = [0, S[0], S[0] + S[1]]
    doff = [0, F0, F0 + F1]

    with tile.TileContext(nc) as tc:
        with (
            tc.tile_pool(name="const", bufs=1) as cpool,
            tc.tile_pool(name="ps512", bufs=4, space="PSUM") as pp5,
            tc.tile_pool(name="ps256", bufs=4, space="PSUM") as pp2,
        ):
            actA = cpool.tile([P, KO, SH + PAD], bf)
            actB = cpool.tile([P, KO, SH + PAD], bf)
            fixout = [cpool.tile([P, KO, FIXW], bf, name=f"fo{f}", tag=f"fo{f}")
                      for f in range(F2)]

            w_sb = []
            for s in range(3):
                row = [cpool.tile([P, KO * LD], bf, name=f"w{s}_{j}", tag=f"w{s}_{j}")
                       for j in range(S[s])]
                w_sb.append(row)

            # pad memsets first: no DMA deps, and the A-pad doubles as the
            # all-zero operand for PE warm-up matmuls during the DMA head
            nc.gpsimd.memset(actA[:, :, SH:SH + PAD], 0.0)
            nc.gpsimd.memset(actB[:, :, SH:SH + PAD], 0.0)

            # head: first chunk's weights and z split by k-block so the k=0
            # matmul can start after ~2 small transfers
            for k in range(KO):
                nc.sync.dma_start(w_sb[0][0][:, k * LD:(k + 1) * LD],
                                  Wv[soff[0]][:, k * LD:(k + 1) * LD])
                nc.sync.dma_start(actA[:, k, 0:CH], zv[:, k, 0:CH])
                if k == 0 and has_bias:
                    # only stage-0 chunk biases are needed on the head chain
                    bias_sb = cpool.tile([P, S_tot * KO], f32)
                    nc.sync.dma_start(bias_sb[:, 0:NCH * KO], Bt[:, 0:NCH * KO])
            H = KO * LD // 2  # k01 | k23 halves of a weight slot
            for j in range(1, NCH):
                nc.sync.dma_start(w_sb[0][j][:, 0:H], Wv[soff[0] + j][:, 0:H])
                nc.sync.dma_start(actA[:, 0:2, j * CH:(j + 1) * CH],
                                  zv[:, 0:2, j * CH:(j + 1) * CH])
                nc.sync.dma_start(w_sb[0][j][:, H:], Wv[soff[0] + j][:, H:])
                nc.sync.dma_start(actA[:, 2:4, j * CH:(j + 1) * CH],
                                  zv[:, 2:4, j * CH:(j + 1) * CH])
            if has_bias:
                nc.sync.dma_start(bias_sb[:, NCH * KO:], Bt[:, NCH * KO:])
            desc_sb = cpool.tile([1, ND], i32)
            nc.sync.dma_start(desc_sb[:], Dt)
            for f in range(F0):
                nc.sync.dma_start(w_sb[0][NCH + f][:, 0:H],
                                  Wv[soff[0] + NCH + f][:, 0:H])
                nc.sync.dma_start(w_sb[0][NCH + f][:, H:],
                                  Wv[soff[0] + NCH + f][:, H:])
            for s in (1, 2):
                for j in range(S[s]):
                    nc.sync.dma_start(w_sb[s][j][:], Wv[soff[s] + j])

            # PE p-state warm-up on the zeroed pad while the head DMAs land
            psw = pp2.tile([P, FIXW], f32, tag="psf", name="psw")

            def dummies(n):
                for i in range(n):
                    nc.tensor.matmul(psw[:, 0:64],
                                     lhsT=actA[:, 0, SH:SH + P],
                                     rhs=actA[:, 0, SH:SH + 64],
                                     start=True, stop=True)

            dummies(WARMUP)

            def evac(m, dst_ap, psum_ap, bias_ap):
                if m % 2 == 0:
                    if has_bias:
                        nc.scalar.activation(dst_ap, psum_ap,
                                             mybir.ActivationFunctionType.Relu,
                                             bias=bias_ap)
                    else:
                        nc.scalar.activation(dst_ap, psum_ap,
                                             mybir.ActivationFunctionType.Relu)
                else:
                    nc.vector.tensor_scalar(dst_ap, psum_ap,
                                            bias_ap if has_bias else 0.0, 0.0,
                                            mybir.AluOpType.add,
                                            mybir.AluOpType.max)

            def chunk_window(s, cur, nxt, j, lo=None, w=None):
                wt = w_sb[s][j]
                lo = j * CH if lo is None else lo
                w = CH if w is None else w
                for m in range(KO):
                    if w == CH:
                        psum = pp5.tile([P, CH], f32, tag="ps", name="psum")
                    else:
                        psum = pp2.tile([P, FIXW], f32, tag="psf", name="psumh")
                    for k in range(KO):
                        nc.tensor.matmul(
                            psum[:],
                            lhsT=wt[:, k * LD + m * P: k * LD + (m + 1) * P],
                            rhs=cur[:, k, lo:lo + w],
                            start=(k == 0), stop=(k == KO - 1))
                    bias_ap = (bias_sb[:, (soff[s] + j) * KO + m:
                                       (soff[s] + j) * KO + m + 1]
                               if has_bias else None)
                    evac(m, nxt[:, m, lo:lo + w], psum[:], bias_ap)
                if s == 2:
                    # chunk rows are pre-fix; fixed rows are merged on host
                    # from the fixO staging written by the stage-2 fixes
                    nc.sync.dma_start(ov[:, :, j * CH:(j + 1) * CH],
                                      nxt[:, :, j * CH:(j + 1) * CH])

            def fix_window(s, cur, nxt, f):
                idx = doff[s] + f
                last = s == 2
                r = nc.values_load(
                    desc_sb[0:1, idx:idx + 1],
                    engines=[PE] if last else [PE, ACT, DVE],
                    min_val=0, max_val=SH - 1,
                    skip_runtime_bounds_check=True)
                wt = w_sb[s][NCH + f]
                fo = fixout[f] if last else None
                for m in range(KO):
                    psum = pp2.tile([P, FIXW], f32, tag="psf", name="psumf")
                    for k in range(KO):
                        nc.tensor.matmul(
                            psum[:],
                            lhsT=wt[:, k * LD + m * P: k * LD + (m + 1) * P],
                            rhs=cur[:, k, ds(r, FIXW)],
                            start=(k == 0), stop=(k == KO - 1))
                    bias_ap = (bias_sb[:, (soff[s] + NCH + f) * KO + m:
                                       (soff[s] + NCH + f) * KO + m + 1]
                               if has_bias else None)
                    dst = fo[:, m, :] if last else nxt[:, m, ds(r, FIXW)]
                    evac(m, dst, psum[:], bias_ap)
                if last:
                    nc.sync.dma_start(fv[:, :, f * FIXW:(f + 1) * FIXW], fo[:])

            for s in range(3):
                cur, nxt = (actA, actB) if s % 2 == 0 else (actB, actA)
                for j in range(NCH):
                    chunk_window(s, cur, nxt, j)
                    if s == 0:
                        # keep the PE p-state ramp pinned across head DMA gaps
                        dummies(GAPFILL[j])
                for f in range(F[s]):
                    fix_window(s, cur, nxt, f)
    nc.compile()
    return nc


def _prep_inputs(z, Ws, bs, stages, rows):
    F = [st[2] for st in stages]
    S = [NCH + f for f in F]
    S_tot = sum(S)
    F_tot = sum(F)
    Wpk_s = [_pack_w(Ws[s]).astype(BF16) for s in range(3)]
    z32 = np.asarray(z, np.float32)

    in_maps = []
    for c in range(NCORES):
        zc = z32[rows[c]] if rows is not None else z32[c * SH:(c + 1) * SH]
        zTc = np.ascontiguousarray(zc.T).astype(BF16)
        slots = []   # (stage, expert)
        for s in range(3):
            chunk_e, fixes, _ = stages[s]
            for j in range(NCH):
                slots.append((s, int(chunk_e[c][j])))
            for r, e in fixes[c]:
                slots.append((s, int(e)))
        Wpk = np.empty((S_tot * P, KO * LD), BF16)
        bias = np.empty((P, S_tot * KO), np.float32)
        for i, (s, e) in enumerate(slots):
            Wpk[i * P:(i + 1) * P] = Wpk_s[s][e]
            bias[:, i * KO:(i + 1) * KO] = bs[s][e].reshape(KO, P).T
        desc = np.zeros((1, max(F_tot, 1)), np.int32)
        i = 0
        for s in range(3):
            _, fixes, _ = stages[s]
            for r, e in fixes[c]:
                desc[0, i] = r
                i += 1
        in_maps.append({
            "zT": zTc,
            "Wpk": Wpk,
            "bias": bias,
            "desc": desc,
        })
    return in_maps


def _kernel_numpy_fallback(z, Ws, bs, ids_all):
    out = np.asarray(z, np.float32)
    for s in range(3):
        nxt = np.empty_like(out)
        ids = ids_all[s]
        for e in range(Ws[s].shape[0]):
            mask = ids == e
            if mask.any():
                nxt[mask] = np.maximum(out[mask] @ Ws[s][e] + bs[s][e], 0.0)
        out = nxt
    return out


def kernel(z, W_dataset, b_dataset, W_assay, b_assay, W_donor, b_donor,
           dataset_ids, assay_ids, donor_ids):
    global LAST_RESULTS
    ids_all = [
        np.asarray(dataset_ids, np.int32),
        np.asarray(assay_ids, np.int32),
        np.asarray(donor_ids, np.int32),
    ]
    Ws = [
        np.ascontiguousarray(np.asarray(W_dataset, np.float32)),
        np.ascontiguousarray(np.asarray(W_assay, np.float32)),
        np.ascontiguousarray(np.asarray(W_donor, np.float32)),
    ]
    bs = [
        np.asarray(b_dataset, np.float32),
        np.asarray(b_assay, np.float32),
        np.asarray(b_donor, np.float32),
    ]
    if any(np.any(np.diff(ids) < 0) for ids in ids_all):
        return _kernel_numpy_fallback(z, Ws, bs, ids_all)

    import os
    rows = None
    if os.environ.get("MOE_ASSIGN", "1") == "1":
        try:
            rows = _assign_rows(ids_all)
        except AssertionError:
            rows = None
    hazard = []
    for s in range(3):
        for c in range(NCORES):
            loc = ids_all[s][rows[c]] if rows is not None \
                else ids_all[s][c * SH:(c + 1) * SH]
            _core_fixes(loc, hazard)
    if hazard:
        return _kernel_numpy_fallback(z, Ws, bs, ids_all)
    stages = _structure(ids_all, rows)
    F = tuple(st[2] for st in stages)
    has_bias = any(np.any(b) for b in bs)
    key = (F, has_bias)
    if key not in _program_cache:
        _program_cache[key] = _build_program(F, has_bias)
    nc = _program_cache[key]
    in_maps = _prep_inputs(z, Ws, bs, stages, rows)
    if not has_bias:
        for m in in_maps:
            m.pop("bias", None)
    res = bass_utils.run_bass_kernel_spmd(nc, in_maps, core_ids=list(range(NCORES)))
    LAST_RESULTS = res

    out = np.empty((N, LD), np.float32)
    for c in range(NCORES):
        blk = res.results[c]["outT"].astype(np.float32).T  # [SH, LD] pre-fix
        fo = res.results[c]["fixO"].astype(np.float32).T   # [F2*FIXW, LD]
        for f, (r, e) in enumerate(stages[2][1][c]):       # ASC order merge
            n = min(FIXW, SH - r)
            blk[r:r + n] = fo[f * FIXW:f * FIXW + n]
        if rows is not None:
            out[rows[c]] = blk
        else:
            out[c * SH:(c + 1) * SH] = blk
    return out


# revision 54
# speedup vs baseline: 1.0148x; 1.0148x over previous
"""Trainium2 Bass kernel for nn_DFVAE (3-stage MoE routing with sorted ids).

Static chunk-grid strategy (N=16384, LD=512, experts (8,6,16), 8 cores,
contiguous row shards, bf16 end-to-end):
  - Per (core, stage): 4 STATIC 512-row chunk windows at rows 512j (expert =
    run covering the chunk start), plus F_s dynamic 256-wide "fix" windows
    that rewrite rows between an unaligned run start and the next chunk
    boundary (<=2 fixes per boundary).  Fix windows may spill past row 2048
    into a 256-row pad of the activation tile (memset once, never stored).
  - Weights are host-packed PER CORE in window-slot order (bf16, lhsT
    layout), so every matmul lhsT is a STATIC SBUF address (PE lhsT cannot
    take register offsets).  Only fix windows use dynamic row offsets
    (values_load on PE/ACT/DVE).
  - Activations bf16 in two ping-pong SBUF tiles (A->B->A->B); static APs
    keep Tile's dependency tracking precise so DMA/compute pipeline.
  - z loaded in 4 per-chunk DMAs (pipelined head); output stored per chunk.
  - PSUM evacuation (bias add + relu) split between ACT (m even) and DVE
    (m odd).
"""
import numpy as np
import ml_dtypes

import concourse.mybir as mybir
import concourse.tile as tile
from concourse import bacc, bass_utils
from concourse.bass import ds

N = 16384
LD = 512
NCORES = 8
SH = N // NCORES      # 2048 rows per core
P = 128
KO = LD // P          # 4 contraction/feature subtiles
CH = 512              # static chunk rows
NCH = SH // CH        # 4 chunks per core
FIXW = 256            # fix window rows
PAD = 256             # activation tile pad rows (fix spill)
STAGE_E = (8, 6, 16)

BF16 = ml_dtypes.bfloat16

LAST_RESULTS = None  # test harness reads exec_time_ns off this

_program_cache = {}


def _core_fixes(loc, hazard=None):
    """Fix list [(start, expert)...] for one core's id vector.

    A single fix spills up to FIXW rows past its run; that is only rewritten
    by a later fix when the following run starts unaligned.  A run that ends
    exactly on a chunk boundary (next run aligned, no fix) would leave the
    spill corrupt -- flag it so the caller can fall back.
    """
    starts = np.flatnonzero(np.diff(loc)) + 1
    fl = []
    for i, bp in enumerate(starts):
        bp = int(bp)
        if bp % CH == 0:
            continue
        run_end = int(starts[i + 1]) if i + 1 < len(starts) else SH
        cover_end = min(run_end, (bp // CH + 1) * CH)
        if cover_end <= bp:
            continue
        e = int(loc[bp])
        ln = cover_end - bp
        if ln <= FIXW:
            fl.append((bp, e))
            if (hazard is not None and ln < FIXW and run_end % CH == 0
                    and run_end < SH):
                hazard.append(bp)
        else:
            fl.append((bp, e))
            fl.append((cover_end - FIXW, e))
    fl.sort()
    return fl


def _assign_rows(ids_all):
    """Order-preserving row->core assignment that steers breakpoint offsets.

    Cores are built from the global stream; core c may defer a tail slice of
    its first atomic block to core c+1 (prepended there).  Removing g rows
    early shifts every later breakpoint left by g (mod 512), turning 2-fix
    boundaries (offset < 256) into 1-fix ones; the deferred rows create one
    junction boundary in the next core, whose offset we also control.
    """
    trip = ids_all[0].astype(np.int64) * 10000 + ids_all[1] * 100 + ids_all[2]
    block_of = np.cumsum(np.diff(trip, prepend=trip[0]) != 0)

    # beam over (donor block, donation size); score = per-stage max fixes
    # across cores (the SPMD shape cost), then total fixes
    beam = [(0, 0, np.empty(0, np.int64), (0, 0, 0), 0, [])]
    for c in range(NCORES):
        nxt_states = []
        for _, cursor, bag, fmax, ftot, rs in beam:
            b = len(bag)
            # candidate donations: (hi, g) = remove rows [hi-g, hi) where
            # [.., hi) is the in-range tail of some atomic block
            cands = {(0, 0)}
            if c < NCORES - 1:
                base_end = cursor + SH - b
                # block portions fully inside the base range
                blks = block_of[cursor:base_end]
                ends = np.flatnonzero(np.diff(blks)) + 1  # block ends (local)
                los = np.concatenate([[0], ends])
                his = np.concatenate([ends, [SH - b]])
                # breakpoint repair targets from the unshifted layout
                loc0 = np.concatenate([bag, np.arange(cursor, base_end)])
                wants = set()
                for s in range(3):
                    loc = ids_all[s][loc0]
                    for bp in (np.flatnonzero(np.diff(loc)) + 1):
                        off = int(bp) % CH
                        for tgt in (0, FIXW, 384):
                            gg = (tgt - off) % CH
                            if gg:
                                wants.add((int(bp), gg))
                for lo, hi in zip(los.tolist(), his.tolist()):
                    avail = hi - lo
                    glo = cursor + hi  # global end of this block portion
                    for bp, gg in wants:
                        # donor at/before the repaired breakpoint
                        if gg < avail and b + hi <= bp + gg:
                            cands.add((glo, gg))
                    if avail > 256:
                        cands.add((glo, 256))
            for hi, g in sorted(cands):
                take = SH - b + g
                if cursor + take > N or (c == NCORES - 1 and
                                         cursor + take != N):
                    continue
                if g:
                    idx = np.concatenate([
                        bag,
                        np.arange(cursor, hi - g),
                        np.arange(hi, cursor + take),
                    ])
                else:
                    idx = np.concatenate([bag,
                                          np.arange(cursor, cursor + take)])
                if len(idx) != SH:
                    continue
                fcs = [len(_core_fixes(ids_all[s][idx])) for s in range(3)]
                nf = (max(fmax[0], fcs[0]), max(fmax[1], fcs[1]),
                      max(fmax[2], fcs[2]))
                key = (nf[0] + nf[1] + nf[2], nf[2], ftot + sum(fcs))
                nbag = np.arange(hi - g, hi) if g else np.empty(0, np.int64)
                nxt_states.append((key, cursor + take, nbag,
                                   nf, ftot + sum(fcs), rs + [idx]))
        nxt_states.sort(key=lambda st: st[0])
        seen = set()
        beam = []
        zero_path = None
        for st in nxt_states:
            sk = (st[1], len(st[2]), int(st[2][0]) if len(st[2]) else -1)
            if st[1] == (c + 1) * SH and len(st[2]) == 0:
                zero_path = zero_path or st
            if sk in seen:
                continue
            seen.add(sk)
            beam.append(st)
            if len(beam) >= 24:
                break
        if zero_path is not None and zero_path not in beam:
            beam.append(zero_path)
    key, cursor, bag, fmax, ftot, rows = beam[0]
    assert cursor == N and len(bag) == 0, (cursor, len(bag))
    if sum(fmax) >= sum(
            max(len(_core_fixes(ids_all[s][c * SH:(c + 1) * SH]))
                for c in range(NCORES)) for s in range(3)):
        return None  # no better than contiguous
    return rows


def _structure(ids_all, rows=None):
    """Per stage: (chunk_experts[8][4], fixes[8]=[(start,expert)...], F)."""
    out = []
    for s in range(3):
        ids = ids_all[s]
        chunk_e = np.zeros((NCORES, NCH), np.int64)
        fixes = []
        for c in range(NCORES):
            loc = ids[rows[c]] if rows is not None else ids[c * SH:(c + 1) * SH]
            for j in range(NCH):
                chunk_e[c, j] = loc[j * CH]
            fixes.append(_core_fixes(loc))
        F = max(len(f) for f in fixes)
        for c in range(NCORES):
            fl = fixes[c]
            filler = fl[-1] if fl else (0, int(chunk_e[c][0]))
            while len(fl) < F:
                fl.append(filler)
        out.append((chunk_e, fixes, F))
    return out


def _pack_w(W):
    """[E, LD, LD] -> [E, P, KO*LD] lhsT layout (k-major blocks)."""
    E = W.shape[0]
    return np.ascontiguousarray(
        W.reshape(E, KO, P, LD).transpose(0, 2, 1, 3).reshape(E, P, KO * LD))


WARMUP = 24
GAPFILL = (0, 0, 0, 0)


def _build_program(F, has_bias=True):
    F0, F1, F2 = F
    S = [NCH + F0, NCH + F1, NCH + F2]
    S_tot = sum(S)
    F_tot = F0 + F1 + F2
    nc = bacc.Bacc("TRN2", target_bir_lowering=False, debug=False,
                   enable_asserts=False, num_devices=NCORES)
    bf = mybir.dt.bfloat16
    f32 = mybir.dt.float32
    i32 = mybir.dt.int32
    PE = mybir.EngineType.PE
    ACT = mybir.EngineType.Activation
    DVE = mybir.EngineType.DVE

    ND = max(F_tot, 1)
    zT = nc.dram_tensor("zT", [LD, SH], bf, kind="ExternalInput").ap()
    Wt = nc.dram_tensor("Wpk", [S_tot * P, KO * LD], bf, kind="ExternalInput").ap()
    Bt = (nc.dram_tensor("bias", [P, S_tot * KO], f32, kind="ExternalInput").ap()
          if has_bias else None)
    Dt = nc.dram_tensor("desc", [1, ND], i32, kind="ExternalInput").ap()
    Ot = nc.dram_tensor("outT", [LD, SH], bf, kind="ExternalOutput").ap()
    # stage-2 fix results land in disjoint static staging; host merges them
    Ft = nc.dram_tensor("fixO", [LD, max(F2, 1) * FIXW], bf,
                        kind="ExternalOutput").ap()

    zv = zT.rearrange("(ko p) r -> p ko r", p=P)
    ov = Ot.rearrange("(ko p) r -> p ko r", p=P)
    fv = Ft.rearrange("(ko p) r -> p ko r", p=P)
    Wv = Wt.rearrange("(s p) c -> s p c", p=P)

    soff = [0, S[0], S[0] + S[1]]
    doff = [0, F0, F0 + F1]

    with tile.TileContext(nc) as tc:
        with (
            tc.tile_pool(name="const", bufs=1) as cpool,
            tc.tile_pool(name="ps512", bufs=4, space="PSUM") as pp5,
            tc.tile_pool(name="ps256", bufs=4, space="PSUM") as pp2,
        ):
            actA = cpool.tile([P, KO, SH + PAD], bf)
            actB = cpool.tile([P, KO, SH + PAD], bf)
            fixout = [cpool.tile([P, KO, FIXW], bf, name=f"fo{f}", tag=f"fo{f}")
                      for f in range(F2)]

            w_sb = []
            for s in range(3):
                row = [cpool.tile([P, KO * LD], bf, name=f"w{s}_{j}", tag=f"w{s}_{j}")
                       for j in range(S[s])]
                w_sb.append(row)

            # pad memsets first: no DMA deps, and the A-pad doubles as the
            # all-zero operand for PE warm-up matmuls during the DMA head
            nc.gpsimd.memset(actA[:, :, SH:SH + PAD], 0.0)
            nc.gpsimd.memset(actB[:, :, SH:SH + PAD], 0.0)

            # head: first chunk's weights and z split by k-block so the k=0
            # matmul can start after ~2 small transfers
            # halves everywhere: 0.73us transfers stay just above the
            # ~0.65us per-DMA issue chain, so the DMA engine never idles
            H = KO * LD // 2  # k01 | k23 halves of a weight slot
            for h in range(2):
                nc.sync.dma_start(w_sb[0][0][:, h * H:(h + 1) * H],
                                  Wv[soff[0]][:, h * H:(h + 1) * H])
                nc.sync.dma_start(actA[:, 2 * h:2 * h + 2, 0:CH],
                                  zv[:, 2 * h:2 * h + 2, 0:CH])
                if h == 0 and has_bias:
                    # only stage-0 chunk biases are needed on the head chain
                    bias_sb = cpool.tile([P, S_tot * KO], f32)
                    nc.sync.dma_start(bias_sb[:, 0:NCH * KO], Bt[:, 0:NCH * KO])
            for j in range(1, NCH):
                nc.sync.dma_start(w_sb[0][j][:, 0:H], Wv[soff[0] + j][:, 0:H])
                nc.sync.dma_start(actA[:, 0:2, j * CH:(j + 1) * CH],
                                  zv[:, 0:2, j * CH:(j + 1) * CH])
                nc.sync.dma_start(w_sb[0][j][:, H:], Wv[soff[0] + j][:, H:])
                nc.sync.dma_start(actA[:, 2:4, j * CH:(j + 1) * CH],
                                  zv[:, 2:4, j * CH:(j + 1) * CH])
            if has_bias:
                nc.sync.dma_start(bias_sb[:, NCH * KO:], Bt[:, NCH * KO:])
            for f in range(F0):
                nc.sync.dma_start(w_sb[0][NCH + f][:, 0:H],
                                  Wv[soff[0] + NCH + f][:, 0:H])
                nc.sync.dma_start(w_sb[0][NCH + f][:, H:],
                                  Wv[soff[0] + NCH + f][:, H:])
            desc_sb = cpool.tile([1, ND], i32)
            nc.sync.dma_start(desc_sb[:], Dt)
            for s in (1, 2):
                for j in range(S[s]):
                    nc.sync.dma_start(w_sb[s][j][:], Wv[soff[s] + j])

            # PE p-state warm-up on the zeroed pad while the head DMAs land
            psw = pp2.tile([P, FIXW], f32, tag="psf", name="psw")

            def dummies(n):
                for i in range(n):
                    nc.tensor.matmul(psw[:, 0:64],
                                     lhsT=actA[:, 0, SH:SH + P],
                                     rhs=actA[:, 0, SH:SH + 64],
                                     start=True, stop=True)

            dummies(WARMUP)

            def evac(m, dst_ap, psum_ap, bias_ap):
                if m % 2 == 0:
                    if has_bias:
                        nc.scalar.activation(dst_ap, psum_ap,
                                             mybir.ActivationFunctionType.Relu,
                                             bias=bias_ap)
                    else:
                        nc.scalar.activation(dst_ap, psum_ap,
                                             mybir.ActivationFunctionType.Relu)
                else:
                    nc.vector.tensor_scalar(dst_ap, psum_ap,
                                            bias_ap if has_bias else 0.0, 0.0,
                                            mybir.AluOpType.add,
                                            mybir.AluOpType.max)

            def chunk_window(s, cur, nxt, j, lo=None, w=None):
                wt = w_sb[s][j]
                lo = j * CH if lo is None else lo
                w = CH if w is None else w
                for m in range(KO):
                    if w == CH:
                        psum = pp5.tile([P, CH], f32, tag="ps", name="psum")
                    else:
                        psum = pp2.tile([P, FIXW], f32, tag="psf", name="psumh")
                    for k in range(KO):
                        nc.tensor.matmul(
                            psum[:],
                            lhsT=wt[:, k * LD + m * P: k * LD + (m + 1) * P],
                            rhs=cur[:, k, lo:lo + w],
                            start=(k == 0), stop=(k == KO - 1))
                    bias_ap = (bias_sb[:, (soff[s] + j) * KO + m:
                                       (soff[s] + j) * KO + m + 1]
                               if has_bias else None)
                    evac(m, nxt[:, m, lo:lo + w], psum[:], bias_ap)
                if s == 2:
                    # chunk rows are pre-fix; fixed rows are merged on host
                    # from the fixO staging written by the stage-2 fixes
                    nc.sync.dma_start(ov[:, :, j * CH:(j + 1) * CH],
                                      nxt[:, :, j * CH:(j + 1) * CH])

            def fix_window(s, cur, nxt, f):
                idx = doff[s] + f
                last = s == 2
                r = nc.values_load(
                    desc_sb[0:1, idx:idx + 1],
                    engines=[PE] if last else [PE, ACT, DVE],
                    min_val=0, max_val=SH - 1,
                    skip_runtime_bounds_check=True)
                wt = w_sb[s][NCH + f]
                fo = fixout[f] if last else None
                for m in range(KO):
                    psum = pp2.tile([P, FIXW], f32, tag="psf", name="psumf")
                    for k in range(KO):
                        nc.tensor.matmul(
                            psum[:],
                            lhsT=wt[:, k * LD + m * P: k * LD + (m + 1) * P],
                            rhs=cur[:, k, ds(r, FIXW)],
                            start=(k == 0), stop=(k == KO - 1))
                    bias_ap = (bias_sb[:, (soff[s] + NCH + f) * KO + m:
                                       (soff[s] + NCH + f) * KO + m + 1]
                               if has_bias else None)
                    dst = fo[:, m, :] if last else nxt[:, m, ds(r, FIXW)]
                    evac(m, dst, psum[:], bias_ap)
                if last:
                    nc.sync.dma_start(fv[:, :, f * FIXW:(f + 1) * FIXW], fo[:])

            for s in range(3):
                cur, nxt = (actA, actB) if s % 2 == 0 else (actB, actA)
                for j in range(NCH):
                    chunk_window(s, cur, nxt, j)
                    if s == 0:
                        # keep the PE p-state ramp pinned across head DMA gaps
                        dummies(GAPFILL[j])
                for f in range(F[s]):
                    fix_window(s, cur, nxt, f)
    nc.compile()
    return nc


def _prep_inputs(z, Ws, bs, stages, rows):
    F = [st[2] for st in stages]
    S = [NCH + f for f in F]
    S_tot = sum(S)
    F_tot = sum(F)
    Wpk_s = [_pack_w(Ws[s]).astype(BF16) for s in range(3)]
    z32 = np.asarray(z, np.float32)

    in_maps = []
    for c in range(NCORES):
        zc = z32[rows[c]] if rows is not None else z32[c * SH:(c + 1) * SH]
        zTc = np.ascontiguousarray(zc.T).astype(BF16)
        slots = []   # (stage, expert)
        for s in range(3):
            chunk_e, fixes, _ = stages[s]
            for j in range(NCH):
                slots.append((s, int(chunk_e[c][j])))
            for r, e in fixes[c]:
                slots.append((s, int(e)))
        Wpk = np.empty((S_tot * P, KO * LD), BF16)
        bias = np.empty((P, S_tot * KO), np.float32)
        for i, (s, e) in enumerate(slots):
            Wpk[i * P:(i + 1) * P] = Wpk_s[s][e]
            bias[:, i * KO:(i + 1) * KO] = bs[s][e].reshape(KO, P).T
        desc = np.zeros((1, max(F_tot, 1)), np.int32)
        i = 0
        for s in range(3):
            _, fixes, _ = stages[s]
            for r, e in fixes[c]:
                desc[0, i] = r
                i += 1
        in_maps.append({
            "zT": zTc,
            "Wpk": Wpk,
            "bias": bias,
            "desc": desc,
        })
    return in_maps


def _kernel_numpy_fallback(z, Ws, bs, ids_all):
    out = np.asarray(z, np.float32)
    for s in range(3):
        nxt = np.empty_like(out)
        ids = ids_all[s]
        for e in range(Ws[s].shape[0]):
            mask = ids == e
            if mask.any():
                nxt[mask] = np.maximum(out[mask] @ Ws[s][e] + bs[s][e], 0.0)
        out = nxt
    return out


def kernel(z, W_dataset, b_dataset, W_assay, b_assay, W_donor, b_donor,
           dataset_ids, assay_ids, donor_ids):
    global LAST_RESULTS
    ids_all = [
        np.asarray(dataset_ids, np.int32),
        np.asarray(assay_ids, np.int32),
        np.asarray(donor_ids, np.int32),
    ]
    Ws = [
        np.ascontiguousarray(np.asarray(W_dataset, np.float32)),
        np.ascontiguousarray(np.asarray(W_assay, np.float32)),
        np.ascontiguousarray(np.asarray(W_donor, np.float32)),
    ]
    bs = [
        np.asarray(b_dataset, np.float32),
        np.asarray(b_assay, np.float32),
        np.asarray(b_donor, np.float32),
    ]
    if any(np.any(np.diff(ids) < 0) for ids in ids_all):
        return _kernel_numpy_fallback(z, Ws, bs, ids_all)

    import os
    rows = None
    if os.environ.get("MOE_ASSIGN", "1") == "1":
        try:
            rows = _assign_rows(ids_all)
        except AssertionError:
            rows = None
    hazard = []
    for s in range(3):
        for c in range(NCORES):
            loc = ids_all[s][rows[c]] if rows is not None \
                else ids_all[s][c * SH:(c + 1) * SH]
            _core_fixes(loc, hazard)
    if hazard:
        return _kernel_numpy_fallback(z, Ws, bs, ids_all)
    stages = _structure(ids_all, rows)
    F = tuple(st[2] for st in stages)
    has_bias = any(np.any(b) for b in bs)
    key = (F, has_bias)
    if key not in _program_cache:
        _program_cache[key] = _build_program(F, has_bias)
    nc = _program_cache[key]
    in_maps = _prep_inputs(z, Ws, bs, stages, rows)
    if not has_bias:
        for m in in_maps:
            m.pop("bias", None)
    res = bass_utils.run_bass_kernel_spmd(nc, in_maps, core_ids=list(range(NCORES)))
    LAST_RESULTS = res

    out = np.empty((N, LD), np.float32)
    for c in range(NCORES):
        blk = res.results[c]["outT"].astype(np.float32).T  # [SH, LD] pre-fix
        fo = res.results[c]["fixO"].astype(np.float32).T   # [F2*FIXW, LD]
        for f, (r, e) in enumerate(stages[2][1][c]):       # ASC order merge
            n = min(FIXW, SH - r)
            blk[r:r + n] = fo[f * FIXW:f * FIXW + n]
        if rows is not None:
            out[rows[c]] = blk
        else:
            out[c * SH:(c + 1) * SH] = blk
    return out
